# revision 1
# baseline (speedup 1.0000x reference)
"""Causal multi-head attention (B=2, S=2048, D=2048, H=16) on 8 TRN2 cores.

Sharding: core c = (batch b = c//4, head-group r = c%4 -> heads 4r..4r+3).
Per core: project q/k/v for its 4 heads over all tokens (fp32r matmuls),
RoPE, exact-causal attention in transposed-score layout (scoresT[keys, q]
via lhsT=k_fm, rhs=q_fm; z[dv, q] via lhsT=v_tokmajor, rhs=expT -- no
on-chip transposes), output-projection partials, then a per-512-token-block
ReduceScatter across the 4 cores of each batch group.

Numerics: all matmuls in fp32r (1s+8e+11m, inputs pre-rounded host-side or
rounded by the producing engine op), fp32 PSUM accumulation, exp on ACT,
softmax without max-subtraction (scores are O(1) here; no overflow).
"""
import sys

sys.path.insert(0, "/opt/trn_rl_repo")

from contextlib import ExitStack

import numpy as np

import concourse.bass as bass  # noqa: F401  (bass must import before tile)
import concourse.mybir as mybir
import concourse.tile as tile
from concourse import bacc
from concourse.bass_utils import run_bass_kernel_spmd

dt = mybir.dt
P = 128
D = 2048
N_HEAD = 16
DH = 128
HPC = 4            # heads per core
ROPE_BASE = 10000.0
GROUPS = [[0, 1, 2, 3], [4, 5, 6, 7]]


def _round_fp32r(x: np.ndarray) -> np.ndarray:
    """RNE round fp32 to the fp32r (11-bit mantissa) grid; returns float32."""
    b = np.ascontiguousarray(x, dtype=np.float32).view(np.uint32)
    b = b + np.uint32(0x7FF) + ((b >> np.uint32(12)) & np.uint32(1))
    b = b & np.uint32(0xFFFFF000)
    return b.view(np.float32)


def _build(S: int):
    NP = S // 512  # token phases
    f32, f32r = dt.float32, dt.float32r
    nc = bacc.Bacc(None, target_bir_lowering=False, num_devices=8)

    xT = nc.declare_dram_parameter("xT", [D, S], f32r, isOutput=False)
    wqT = nc.declare_dram_parameter("wqT", [D, 512], f32r, isOutput=False)
    wkT = nc.declare_dram_parameter("wkT", [D, 512], f32r, isOutput=False)
    wvT = nc.declare_dram_parameter("wvT", [D, 512], f32r, isOutput=False)
    woT = nc.declare_dram_parameter("woT", [512, D], f32r, isOutput=False)
    cosk = nc.declare_dram_parameter("cosk", [P, S], f32, isOutput=False)
    sink = nc.declare_dram_parameter("sink", [P, S], f32, isOutput=False)
    masks = nc.declare_dram_parameter("masks", [P, 896], f32, isOutput=False)
    permm = nc.declare_dram_parameter("permm", [P, P], f32r, isOutput=False)
    out_sh = nc.declare_dram_parameter("out_sh", [NP, 512, 512], f32,
                                       isOutput=True)

    rs_in = [nc.dram_tensor(f"rs_in{T}", [D, 512], f32) for T in range(NP)]
    rs_out = [nc.dram_tensor(f"rs_out{T}", [512, 512], f32) for T in range(NP)]
    rs_in_h = [nc.dram_tensor(f"rs_in_h{i}", [D, 256], f32) for i in range(2)]
    rs_out_h = [nc.dram_tensor(f"rs_out_h{i}", [512, 256], f32) for i in range(2)]

    xT_r = xT.rearrange("(kt p) s -> p kt s", p=P)

    with tile.TileContext(nc) as tc, ExitStack() as ctx:
        const = ctx.enter_context(tc.tile_pool(name="const", bufs=1))
        kvres = ctx.enter_context(tc.tile_pool(name="kvres", bufs=1))
        xp = ctx.enter_context(tc.tile_pool(name="xp", bufs=8))
        wqkp = ctx.enter_context(tc.tile_pool(name="wqkp", bufs=7))
        wop = ctx.enter_context(tc.tile_pool(name="wop", bufs=7))
        wvp = ctx.enter_context(tc.tile_pool(name="wvp", bufs=4))
        rp = ctx.enter_context(tc.tile_pool(name="rp", bufs=3))
        qp = ctx.enter_context(tc.tile_pool(name="qp", bufs=2))
        zp = ctx.enter_context(tc.tile_pool(name="zp", bufs=1))
        ep = ctx.enter_context(tc.tile_pool(name="ep", bufs=3))
        dp = ctx.enter_context(tc.tile_pool(name="dp", bufs=2))
        op_ = ctx.enter_context(tc.tile_pool(name="op", bufs=2))
        tabp = ctx.enter_context(tc.tile_pool(name="tabp", bufs=1))
        pp = ctx.enter_context(tc.tile_pool(name="pp", bufs=4, space="PSUM"))
        ps_wo = ctx.enter_context(tc.tile_pool(name="ps_wo", bufs=1, space="PSUM"))
        ps_sc = ctx.enter_context(tc.tile_pool(name="ps_sc", bufs=2, space="PSUM"))
        ps_z = ctx.enter_context(tc.tile_pool(name="ps_z", bufs=1, space="PSUM"))

        ones128 = const.tile([P, P], f32)
        masks_sb = const.tile([P, 896], f32)
        permm_sb = const.tile([P, P], f32r)
        nc.sync.dma_start(out=permm_sb, in_=permm[:, :])

        def load_consts():
            nc.vector.memset(ones128, 1.0)
            nc.sync.dma_start(out=masks_sb, in_=masks[:, :])

        # persistent K (feature-major) and V (token-major) per 512-token phase
        k_sbs = [kvres.tile([P, HPC, 512], f32r, tag=f"k_sb{T}", name=f"k_sb{T}")
                 for T in range(NP)]
        v_sbs = [kvres.tile([P, 4, 512], f32r, tag=f"v_sb{T}", name=f"v_sb{T}")
                 for T in range(NP)]

        q_sbs = {}
        z_sbs = {}

        def proj_phase(T):
            tok = slice(512 * T, 512 * (T + 1))
            _mark(nc, f"T{T}.xload")

            x_pairs = {}

            def x_load(pi):
                if pi not in x_pairs:
                    xt = xp.tile([P, 2, 512], f32r, tag="x_pair",
                                 name=f"x_{T}_{pi}")
                    nc.sync.dma_start(out=xt,
                                      in_=xT_r[:, 2 * pi:2 * pi + 2, tok])
                    x_pairs[pi] = xt
                return x_pairs[pi]

            def x_kd(kd):
                return x_load(kd // 2)[:, kd % 2, :]

            # ---- Q / K projections + RoPE ----
            q_sb = qp.tile([P, HPC, 512], f32r, tag="q_sb", name=f"q_sb{T}")
            q_sbs[T] = q_sb
            tabs = {}
            for wt, ctab, stab, is_q in ((wqT, cosk, sink, True),
                                         (wkT, cosk, sink, False)):
                _mark(nc, f"T{T}." + ("qproj" if is_q else "kproj"))
                psl = [pp.tile([P, 512], f32, tag="pp", name=f"psqk{T}{is_q}{h}")
                       for h in range(HPC)]
                for kd in range(16):
                    if is_q and kd % 2 == 0:
                        x_load(kd // 2)
                    w_t = wqkp.tile([P, 512], f32r, tag="w_t")
                    nc.sync.dma_start(out=w_t, in_=wt[P * kd:P * (kd + 1), :])
                    for h in range(HPC):
                        nc.tensor.matmul(psl[h][:],
                                         lhsT=w_t[:, P * h:P * (h + 1)],
                                         rhs=x_kd(kd),
                                         start=(kd == 0), stop=(kd == 15))
                if not tabs:
                    ct = tabp.tile([P, 512], f32, tag="ck", name=f"ct{T}")
                    nc.sync.dma_start(out=ct, in_=ctab[:, tok])
                    st_ = tabp.tile([P, 512], f32, tag="sk", name=f"st{T}")
                    nc.sync.dma_start(out=st_, in_=stab[:, tok])
                    tabs["c"], tabs["s"] = ct, st_
                ct, st_ = tabs["c"], tabs["s"]
                for h in range(HPC):
                    tmp = rp.tile([P, 512], f32r, tag="tmp")
                    if is_q:   # fold the 1/sqrt(Dh) score scale into q
                        nc.vector.tensor_scalar_mul(tmp[:], psl[h][:],
                                                    float(DH) ** -0.5)
                    else:
                        nc.vector.tensor_copy(tmp[:], psl[h][:])
                    ps_rot = ps_wo.tile([P, 512], f32, tag="ps_o",
                                        name=f"ps_rot{T}{is_q}{h}")
                    nc.tensor.matmul(ps_rot[:], lhsT=permm_sb[:], rhs=tmp[:],
                                     start=True, stop=True)
                    t1 = rp.tile([P, 512], f32, tag="t1")
                    nc.vector.tensor_mul(t1[:], tmp[:].bitcast(f32), ct[:])
                    swp = rp.tile([P, 512], f32, tag="swp")
                    nc.vector.tensor_mul(swp[:], ps_rot[:], st_[:])
                    dst = q_sb[:, h, :] if is_q else k_sbs[T][:, h, :]
                    nc.vector.tensor_add(dst, t1[:], swp[:])

            # ---- V projection (token-major), kd-outer for weight reuse ----
            _mark(nc, f"T{T}.vproj")
            psv = [pp.tile([P, 512], f32, tag="pp", name=f"psv{T}{i}")
                   for i in range(4)]
            for kd in range(16):
                wv_t = wvp.tile([P, 512], f32r, tag="wv_t")
                nc.sync.dma_start(out=wv_t, in_=wvT[P * kd:P * (kd + 1), :])
                for i in range(4):
                    xk = x_kd(kd)
                    nc.tensor.matmul(psv[i][:],
                                     lhsT=xk[:, P * i:P * (i + 1)],
                                     rhs=wv_t[:],
                                     start=(kd == 0), stop=(kd == 15))
            for i in range(4):
                nc.vector.tensor_copy(v_sbs[T][:, i, :], psv[i][:])

        def attn_phase(T):
            q_sb = q_sbs.pop(T)
            z_sb = zp.tile([P, HPC, 512], f32r, tag="z_sb", name=f"z_sb{T}")
            nkb = 4 * T + 4
            for h in range(HPC):
                _mark(nc, f"T{T}.attn{h}")
                ps_zt = ps_z.tile([P, 512], f32, tag="ps_z")
                den = dp.tile([P, 512], f32, tag="den")
                for kb in range(nkb):
                    ps_s = ps_sc.tile([P, 512], f32, tag="ps_s")
                    nc.tensor.matmul(
                        ps_s[:],
                        lhsT=k_sbs[kb // 4][:, h, P * (kb % 4):P * (kb % 4 + 1)],
                        rhs=q_sb[:, h, :],
                        start=True, stop=True)
                    et = ep.tile([P, 512], f32r, tag="et")
                    nc.scalar.activation(et[:], ps_s[:],
                                         mybir.ActivationFunctionType.Exp)
                    if kb >= 4 * T:  # diagonal block: causal mask
                        jj = kb - 4 * T
                        em = ep.tile([P, 512], f32r, tag="em")
                        nc.vector.tensor_mul(
                            em[:], et[:].bitcast(f32),
                            masks_sb[:, 384 - 128 * jj:896 - 128 * jj])
                        e_use = em[:]
                    else:
                        e_use = et[:]
                    if kb == 0:
                        nc.vector.tensor_copy(den[:], e_use.bitcast(f32))
                    else:
                        nc.vector.tensor_add(den[:], den[:], e_use.bitcast(f32))
                    nc.tensor.matmul(
                        ps_zt[:],
                        lhsT=v_sbs[kb // 4][:, kb % 4, P * h:P * (h + 1)],
                        rhs=e_use,
                        start=(kb == 0), stop=(kb == nkb - 1))
                # denominator: fold over keys + broadcast in one ones-matmul
                ps_bt = ps_sc.tile([P, 512], f32, tag="ps_s", name=f"ps_bt{T}{h}")
                nc.tensor.matmul(ps_bt[:], lhsT=ones128[:], rhs=den[:],
                                 start=True, stop=True)
                bc_sb = dp.tile([P, 512], f32, tag="bc_sb")
                nc.vector.reciprocal(bc_sb[:], ps_bt[:])
                nc.vector.tensor_mul(z_sb[:, h, :], ps_zt[:], bc_sb[:])
            z_sbs[T] = z_sb

        def wo_phase(T):
            z_sb = z_sbs.pop(T)
            _mark(nc, f"T{T}.wo")
            for mg in range(4):           # m-groups of 4 dout tiles
                wg = [wop.tile([P, 512], f32r, tag="wo_t", name=f"wo{T}{mg}{kd}")
                      for kd in range(HPC)]
                for kd in range(HPC):
                    nc.sync.dma_start(
                        out=wg[kd],
                        in_=woT[P * kd:P * (kd + 1), 512 * mg:512 * (mg + 1)])
                for mi in range(4):
                    m = 4 * mg + mi
                    ps_o = ps_wo.tile([P, 512], f32, tag="ps_o",
                                      name=f"ps_o{T}{m}")
                    for kd in range(HPC):
                        nc.tensor.matmul(ps_o[:],
                                         lhsT=wg[kd][:, P * mi:P * (mi + 1)],
                                         rhs=z_sb[:, kd, :],
                                         start=(kd == 0), stop=(kd == HPC - 1))
                    o_t = op_.tile([P, 512], f32, tag="o_t")
                    nc.scalar.copy(o_t[:], ps_o[:])
                    nc.sync.dma_start(
                        out=rs_in[T][P * m:P * (m + 1), :], in_=o_t[:])
            _mark(nc, f"T{T}.rs0")
            nc.gpsimd.collective_compute(
                "ReduceScatter", mybir.AluOpType.add, replica_groups=GROUPS,
                ins=[rs_in[T][:, :]], outs=[rs_out[T][:, :]])
            nc.sync.dma_start(out=out_sh[T, :, :], in_=rs_out[T][:, :])

        for T in range(NP):
            proj_phase(T)
            if T == 0:
                load_consts()
            if T >= 1:
                attn_phase(T - 1)
                wo_phase(T - 1)
        attn_phase(NP - 1)
        wo_phase(NP - 1)

    nc.compile()
    return nc


REGIONS = []


def _mark(nc, label):
    nid = nc.next_id()  # consumes one id; fine for attribution
    REGIONS.append((label, nid))


_BUILT = {}


def _get_built(S):
    if S not in _BUILT:
        _BUILT[S] = _build(S)
    return _BUILT[S]


def host_inputs(x, w_qkv, w_o):
    """Build the 8 per-core input maps from full inputs."""
    B, S, D_ = x.shape
    scale = 1.0 / np.sqrt(np.float32(DH))

    j = np.arange(0, DH, 2, dtype=np.float32) / DH          # (2j)/Dh, j=0..63
    inv_freq = (1.0 / (ROPE_BASE ** j)).astype(np.float32)  # [64]
    t = np.arange(S, dtype=np.float32)
    freqs = np.outer(inv_freq, t)                            # [64, S]
    emb = np.concatenate([freqs, freqs], axis=0)             # [128, S]
    cos_t = np.cos(emb).astype(np.float32)
    sin_t = np.sin(emb).astype(np.float32)
    cosk_t = np.ascontiguousarray(cos_t)
    sink_t = np.ascontiguousarray(sin_t)
    # rot = R @ q (rotate_half incl. sign); matmul computes lhsT.T @ rhs,
    # so feed R.T: R[d, d+64] = -1 (d<64), R[d, d-64] = +1 (d>=64)
    permm_np = np.zeros((P, P), dtype=np.float32)
    for d_ in range(64):
        permm_np[d_ + 64, d_] = -1.0
        permm_np[d_, d_ + 64] = 1.0

    u_idx = np.arange(896)[None, :]
    k_idx = np.arange(P)[:, None]
    masks_np = (u_idx - 384 >= k_idx).astype(np.float32)  # [128, 896]

    wqkvT = _round_fp32r(w_qkv.T)        # [D, 3D]
    woT_full = _round_fp32r(w_o.T)       # [D(in), D(out)]
    xTb = [_round_fp32r(x[b].T) for b in range(B)]  # [D, S]

    in_maps = []
    for c in range(8):
        b, r = c // 4, c % 4
        in_maps.append({
            "xT": xTb[b],
            "wqT": np.ascontiguousarray(wqkvT[:, 512 * r:512 * (r + 1)]),
            "wkT": np.ascontiguousarray(wqkvT[:, D + 512 * r:D + 512 * (r + 1)]),
            "wvT": np.ascontiguousarray(
                wqkvT[:, 2 * D + 512 * r:2 * D + 512 * (r + 1)]),
            "woT": np.ascontiguousarray(woT_full[512 * r:512 * (r + 1), :]),
            "cosk": cosk_t, "sink": sink_t,
            "masks": masks_np, "permm": permm_np,
        })
    return in_maps


def assemble(results, B, S):
    NP = S // 512
    out = np.empty((B, S, D), dtype=np.float32)
    for c in range(8):
        b, r = c // 4, c % 4
        sh = results[c]["out_sh"]  # [NP, 512(dout), 512(tok)]
        for T in range(NP):
            out[b, 512 * T:512 * (T + 1), 512 * r:512 * (r + 1)] = sh[T].T
    return out


def kernel(x, w_qkv, w_o, _trace=False):
    x = np.asarray(x, dtype=np.float32)
    w_qkv = np.asarray(w_qkv, dtype=np.float32)
    w_o = np.asarray(w_o, dtype=np.float32)
    B, S, _ = x.shape
    nc = _get_built(S)
    in_maps = host_inputs(x, w_qkv, w_o)
    def _run():
        try:
            return run_bass_kernel_spmd(nc, in_maps, list(range(8)),
                                        trace=_trace)
        except ModuleNotFoundError:
            return run_bass_kernel_spmd(nc, in_maps, list(range(8)))

    try:
        res = _run()
    except Exception:
        res = _run()  # transient runtime/readback errors: retry once
    out = assemble(res.results, B, S)
    if _trace:
        return out, res
    return out



# revision 7
# speedup vs baseline: 1.2239x; 1.2239x over previous
"""Causal multi-head attention (B=2, S=2048, D=2048, H=16) on 8 TRN2 cores.

Sharding: core c = (batch b = c//4, head-group r = c%4 -> heads 4r..4r+3).
Per core: project q/k/v for its 4 heads over all tokens (bf16 matmuls, fp32
PSUM), RoPE, exact-causal attention in transposed-score layout (scoresT
[keys, q] via lhsT=k_fm, rhs=q_fm; z[dh, q] via lhsT=v_tokmajor, rhs=e).
Softmax denominator is accumulated on the TensorEngine (ones-matmul) into
the second half of a [128,1024] PSUM tile shared with the z accumulation.
Output projection partials go through a per-phase bf16 ReduceScatter
across the 4 cores of each batch group.

Numerics: bf16 matmul inputs everywhere (fp32 PSUM accumulation), exp on
ACT (no max-subtraction; scores are O(1)), reciprocal in fp32.
"""
import sys

sys.path.insert(0, "/opt/trn_rl_repo")

from contextlib import ExitStack

import ml_dtypes
import numpy as np

import concourse.bass as bass  # noqa: F401  (bass must import before tile)
import concourse.mybir as mybir
import concourse.tile as tile
from concourse import bacc
from concourse.bass_utils import run_bass_kernel_spmd

dt = mybir.dt
BF16 = ml_dtypes.bfloat16
P = 128
D = 2048
N_HEAD = 16
DH = 128
HPC = 4            # heads per core
ROPE_BASE = 10000.0
GROUPS = [[0, 1, 2, 3], [4, 5, 6, 7]]


def _build(S: int):
    NP = S // 512  # token phases
    f32, bf = dt.float32, dt.bfloat16
    nc = bacc.Bacc(None, target_bir_lowering=False, num_devices=8)

    xT = nc.declare_dram_parameter("xT", [P, 16, S], bf, isOutput=False)
    wq = nc.declare_dram_parameter("wq", [P, 16, 512], bf, isOutput=False)
    wk = nc.declare_dram_parameter("wk", [P, 16, 512], bf, isOutput=False)
    wv = nc.declare_dram_parameter("wv", [P, 16, 512], bf, isOutput=False)
    wo = nc.declare_dram_parameter("wo", [P, HPC, 16, P], bf, isOutput=False)
    cosk = nc.declare_dram_parameter("cosk", [P, S], bf, isOutput=False)
    sink = nc.declare_dram_parameter("sink", [P, S], bf, isOutput=False)
    maskk = nc.declare_dram_parameter("maskk", [P, P], bf, isOutput=False)
    permm = nc.declare_dram_parameter("permm", [P, P], bf, isOutput=False)
    out_sh = nc.declare_dram_parameter("out_sh", [NP, 512, 512], bf,
                                       isOutput=True)

    rs_in = [nc.dram_tensor(f"rs_in{T}", [4 * 512, 512], bf)
             for T in range(NP)]
    rs_r = [t.rearrange("(mg mi p) s -> mg p mi s", p=P, mi=4) for t in rs_in]
    rs_out = [nc.dram_tensor(f"rs_out{T}", [512, 512], bf) for T in range(NP)]

    with tile.TileContext(nc) as tc, ExitStack() as ctx:
        const = ctx.enter_context(tc.tile_pool(name="const", bufs=1))
        kvres = ctx.enter_context(tc.tile_pool(name="kvres", bufs=1))
        xp = ctx.enter_context(tc.tile_pool(name="xp", bufs=2))
        qp = ctx.enter_context(tc.tile_pool(name="qp", bufs=2))
        rp = ctx.enter_context(tc.tile_pool(name="rp", bufs=3))
        ep = ctx.enter_context(tc.tile_pool(name="ep", bufs=4))
        bp = ctx.enter_context(tc.tile_pool(name="bp", bufs=2))
        zp = ctx.enter_context(tc.tile_pool(name="zp", bufs=2))
        op_ = ctx.enter_context(tc.tile_pool(name="op", bufs=2))
        pp = ctx.enter_context(tc.tile_pool(name="pp", bufs=2, space="PSUM"))
        sc = ctx.enter_context(tc.tile_pool(name="sc", bufs=2, space="PSUM"))
        zd = ctx.enter_context(tc.tile_pool(name="zd", bufs=2, space="PSUM"))

        wq_sb = const.tile([P, 16, 512], bf, name="wq_sb")
        wk_sb = const.tile([P, 16, 512], bf, name="wk_sb")
        wv_sb = const.tile([P, 16, 512], bf, name="wv_sb")
        wo_sb = const.tile([P, HPC, 16, P], bf, name="wo_sb")
        cos_sb = const.tile([P, S], bf, name="cos_sb")
        sin_sb = const.tile([P, S], bf, name="sin_sb")
        mask_sb = const.tile([P, P], bf, name="mask_sb")
        permm_sb = const.tile([P, P], bf, name="permm_sb")
        ones_sb = const.tile([P, P], bf, name="ones_sb")

        def load_consts():
            # wq is loaded by proj_phase(0) interleaved with the x chunks
            for wt, wt_sb in ((wk, wk_sb), (wv, wv_sb)):
                for half in range(2):
                    ks = slice(8 * half, 8 * half + 8)
                    nc.sync.dma_start(out=wt_sb[:, ks, :], in_=wt[:, ks, :])
                if wt is wk:
                    nc.sync.dma_start(out=cos_sb, in_=cosk[:, :])
                    nc.sync.dma_start(out=sin_sb, in_=sink[:, :])
                    nc.sync.dma_start(out=permm_sb, in_=permm[:, :])
            nc.sync.dma_start(out=wo_sb, in_=wo[:, :, :, :])
            nc.sync.dma_start(out=mask_sb, in_=maskk[:, :])
            nc.vector.memset(ones_sb, 1.0)

        # persistent K (feature-major) and V (token-major) per 512-token phase
        k_sbs = [kvres.tile([P, HPC, 512], bf, tag=f"k_sb{T}", name=f"k_sb{T}")
                 for T in range(NP)]
        v_sbs = [kvres.tile([P, 4, 512], bf, tag=f"v_sb{T}", name=f"v_sb{T}")
                 for T in range(NP)]

        q_sbs = {}
        z_sbs = {}

        def proj_phase(T):
            tok = slice(512 * T, 512 * (T + 1))
            x_t = xp.tile([P, 16, 512], bf, tag="x_t", name=f"x_{T}")
            if T == 0:
                nc.sync.dma_start(out=x_t[:, 0:4, :], in_=xT[:, 0:4, tok])
                for half in range(2):
                    ks = slice(8 * half, 8 * half + 8)
                    nc.sync.dma_start(out=wq_sb[:, ks, :], in_=wq[:, ks, :])
                    nc.sync.dma_start(out=x_t[:, 4 + 6 * half:10 + 6 * half, :],
                                      in_=xT[:, 4 + 6 * half:10 + 6 * half, tok])
                load_consts()
            else:
                nc.sync.dma_start(out=x_t[:, 0:8, :], in_=xT[:, 0:8, tok])
                nc.sync.dma_start(out=x_t[:, 8:16, :], in_=xT[:, 8:16, tok])

            # ---- Q / K projections + RoPE ----
            q_sb = qp.tile([P, HPC, 512], bf, tag="q_sb", name=f"q_sb{T}")
            q_sbs[T] = q_sb
            for wt_sb, is_q in ((wq_sb, True), (wk_sb, False)):
                for h in range(HPC):
                    ps = pp.tile([P, 512], f32, tag="pp")
                    for kd in range(16):
                        nc.tensor.matmul(ps[:],
                                         lhsT=wt_sb[:, kd, P * h:P * (h + 1)],
                                         rhs=x_t[:, kd, :],
                                         start=(kd == 0), stop=(kd == 15))
                    t = rp.tile([P, 512], bf, tag="t")
                    if is_q:   # fold the 1/sqrt(Dh) score scale into q
                        nc.scalar.mul(t[:], ps[:], float(DH) ** -0.5)
                    else:
                        nc.scalar.copy(t[:], ps[:])
                    ps2 = pp.tile([P, 512], f32, tag="pp")
                    nc.tensor.matmul(ps2[:], lhsT=permm_sb[:], rhs=t[:],
                                     start=True, stop=True)
                    u = rp.tile([P, 512], bf, tag="u")
                    nc.vector.tensor_mul(u[:], t[:], cos_sb[:, tok])
                    sw = rp.tile([P, 512], bf, tag="sw")
                    nc.vector.tensor_mul(sw[:], ps2[:], sin_sb[:, tok])
                    dst = q_sb[:, h, :] if is_q else k_sbs[T][:, h, :]
                    nc.vector.tensor_add(dst, u[:], sw[:])

            # ---- V projection (token-major) ----
            for tb in range(4):
                psv = pp.tile([P, 512], f32, tag="pp")
                for kd in range(16):
                    nc.tensor.matmul(psv[:],
                                     lhsT=x_t[:, kd, P * tb:P * (tb + 1)],
                                     rhs=wv_sb[:, kd, :],
                                     start=(kd == 0), stop=(kd == 15))
                nc.scalar.copy(v_sbs[T][:, tb, :], psv[:])

        def attn_phase(T):
            q_sb = q_sbs.pop(T)
            z_sb = zp.tile([P, HPC, 512], bf, tag="z_sb", name=f"z_sb{T}")
            nkb = 4 * T + 4
            for h in range(HPC):
                ps_zd = zd.tile([P, 1024], f32, tag="zd",
                                name=f"ps_zd{T}_{h}")
                for kb in range(nkb):
                    j = kb - 4 * T
                    qlo = P * j if j > 0 else 0
                    qs = slice(qlo, 512)
                    first, last = (kb == 0), (kb == nkb - 1)
                    ps_s = sc.tile([P, 512], f32, tag="sc")
                    nc.tensor.matmul(
                        ps_s[:, qs],
                        lhsT=k_sbs[kb // 4][:, h, P * (kb % 4):P * (kb % 4 + 1)],
                        rhs=q_sb[:, h, qs],
                        start=True, stop=True)
                    e = ep.tile([P, 512], bf, tag="e")
                    nc.scalar.activation(e[:, qs], ps_s[:, qs],
                                         mybir.ActivationFunctionType.Exp)
                    if j >= 0:  # diagonal block: causal triangle mask
                        nc.vector.tensor_mul(e[:, qlo:qlo + P],
                                             e[:, qlo:qlo + P], mask_sb[:])
                    nc.tensor.matmul(
                        ps_zd[:, qs],
                        lhsT=v_sbs[kb // 4][:, kb % 4, P * h:P * (h + 1)],
                        rhs=e[:, qs],
                        start=first, stop=last, skip_group_check=True)
                    nc.tensor.matmul(
                        ps_zd[:, 512 + qlo:1024],
                        lhsT=ones_sb[:],
                        rhs=e[:, qs],
                        start=first, stop=last, skip_group_check=True)
                bc = bp.tile([P, 512], f32, tag="bc")
                nc.vector.reciprocal(bc[:], ps_zd[:, 512:1024])
                nc.vector.tensor_mul(z_sb[:, h, :], ps_zd[:, 0:512], bc[:])
            z_sbs[T] = z_sb

        def wo_phase(T):
            z_sb = z_sbs.pop(T)
            for mg in range(4):
                o_sb = op_.tile([P, 4, 512], bf, tag="o_sb")
                for mi in range(4):
                    m = 4 * mg + mi
                    ps_o = pp.tile([P, 512], f32, tag="pp")
                    for kd in range(HPC):
                        nc.tensor.matmul(ps_o[:],
                                         lhsT=wo_sb[:, kd, m, :],
                                         rhs=z_sb[:, kd, :],
                                         start=(kd == 0), stop=(kd == HPC - 1))
                    nc.vector.tensor_copy(o_sb[:, mi, :], ps_o[:])
                nc.sync.dma_start(out=rs_r[T][mg], in_=o_sb[:])
            nc.gpsimd.collective_compute(
                "ReduceScatter", mybir.AluOpType.add, replica_groups=GROUPS,
                ins=[rs_in[T][:, :]], outs=[rs_out[T][:, :]])
            nc.sync.dma_start(out=out_sh[T, :, :], in_=rs_out[T][:, :])

        for T in range(NP):
            proj_phase(T)
            if T >= 1:
                attn_phase(T - 1)
                wo_phase(T - 1)
        attn_phase(NP - 1)
        wo_phase(NP - 1)

    nc.compile()
    return nc


_BUILT = {}


def _get_built(S):
    if S not in _BUILT:
        _BUILT[S] = _build(S)
    return _BUILT[S]


def _bf16(x: np.ndarray) -> np.ndarray:
    return np.ascontiguousarray(x.astype(BF16))


def host_inputs(x, w_qkv, w_o):
    """Build the 8 per-core input maps from full inputs."""
    B, S, D_ = x.shape

    j = np.arange(0, DH, 2, dtype=np.float32) / DH          # (2j)/Dh, j=0..63
    inv_freq = (1.0 / (ROPE_BASE ** j)).astype(np.float32)  # [64]
    t = np.arange(S, dtype=np.float32)
    freqs = np.outer(inv_freq, t)                            # [64, S]
    emb = np.concatenate([freqs, freqs], axis=0)             # [128, S]
    cos_t = _bf16(np.cos(emb))
    sin_t = _bf16(np.sin(emb))
    # rot = R @ q (rotate_half incl. sign); matmul computes lhsT.T @ rhs,
    # so feed R.T: R[d, d+64] = -1 (d<64), R[d, d-64] = +1 (d>=64)
    permm_np = np.zeros((P, P), dtype=np.float32)
    for d_ in range(64):
        permm_np[d_ + 64, d_] = -1.0
        permm_np[d_, d_ + 64] = 1.0
    permm_np = _bf16(permm_np)

    k_idx = np.arange(P)[:, None]
    q_idx = np.arange(P)[None, :]
    mask_np = _bf16((q_idx >= k_idx).astype(np.float32))     # [128, 128]

    wqkvT = np.asarray(w_qkv, dtype=np.float32).T            # [D, 3D]
    woT_full = np.asarray(w_o, dtype=np.float32).T           # [D(in), D(out)]

    def wslice(r, n):
        ws = wqkvT[:, n * D + 512 * r:n * D + 512 * (r + 1)]  # [D, 512]
        return _bf16(ws.reshape(16, P, 512).transpose(1, 0, 2))

    xTb = [
        _bf16(np.ascontiguousarray(x[b].T).reshape(16, P, S).transpose(1, 0, 2))
        for b in range(B)
    ]

    in_maps = []
    for c in range(8):
        b, r = c // 4, c % 4
        woc = woT_full[512 * r:512 * (r + 1), :]              # [512, D]
        wo_np = _bf16(woc.reshape(HPC, P, 16, P).transpose(1, 0, 2, 3))
        in_maps.append({
            "xT": xTb[b],
            "wq": wslice(r, 0),
            "wk": wslice(r, 1),
            "wv": wslice(r, 2),
            "wo": wo_np,
            "cosk": cos_t, "sink": sin_t,
            "maskk": mask_np, "permm": permm_np,
        })
    return in_maps


def assemble(results, B, S):
    NP = S // 512
    out = np.empty((B, S, D), dtype=np.float32)
    for c in range(8):
        b, r = c // 4, c % 4
        sh = np.asarray(results[c]["out_sh"]).astype(np.float32)
        for T in range(NP):
            out[b, 512 * T:512 * (T + 1), 512 * r:512 * (r + 1)] = sh[T].T
    return out


def kernel(x, w_qkv, w_o, _trace=False):
    x = np.asarray(x, dtype=np.float32)
    w_qkv = np.asarray(w_qkv, dtype=np.float32)
    w_o = np.asarray(w_o, dtype=np.float32)
    B, S, _ = x.shape
    nc = _get_built(S)
    in_maps = host_inputs(x, w_qkv, w_o)

    def _run():
        try:
            return run_bass_kernel_spmd(nc, in_maps, list(range(8)),
                                        trace=_trace)
        except ModuleNotFoundError:
            return run_bass_kernel_spmd(nc, in_maps, list(range(8)))

    try:
        res = _run()
    except Exception:
        res = _run()  # transient runtime/readback errors: retry once
    out = assemble(res.results, B, S)
    if _trace:
        return out, res
    return out


# revision 24
# speedup vs baseline: 1.2672x; 1.0354x over previous
"""Causal multi-head attention (B=2, S=2048, D=2048, H=16) on 8 TRN2 cores.

Sharding: core c = (batch b = c//4, head-group r = c%4 -> heads 4r..4r+3).
Per core: project q/k/v for its 4 heads over all tokens (bf16 matmuls, fp32
PSUM), RoPE, exact-causal attention in transposed-score layout (scoresT
[keys, q] via lhsT=k_fm, rhs=q_fm; z[dh, q] via lhsT=v_tokmajor, rhs=e).
Softmax denominator is accumulated on the TensorEngine (ones-matmul) into
the second half of a [128,1024] PSUM tile shared with the z accumulation.
Output projection partials go through a per-phase bf16 ReduceScatter
across the 4 cores of each batch group.

Numerics: bf16 matmul inputs everywhere (fp32 PSUM accumulation), exp on
ACT (no max-subtraction; scores are O(1)), reciprocal in fp32.
"""
import sys

sys.path.insert(0, "/opt/trn_rl_repo")

from contextlib import ExitStack

import ml_dtypes
import numpy as np

import concourse.bass as bass  # noqa: F401  (bass must import before tile)
import concourse.mybir as mybir
import concourse.tile as tile
from concourse import bacc
from concourse.bass_utils import run_bass_kernel_spmd

dt = mybir.dt
BF16 = ml_dtypes.bfloat16
P = 128
D = 2048
N_HEAD = 16
DH = 128
HPC = 4            # heads per core
ROPE_BASE = 10000.0
GROUPS = [[0, 1, 2, 3], [4, 5, 6, 7]]


def _build(S: int):
    NP = S // 512  # token phases
    f32, bf = dt.float32, dt.bfloat16
    nc = bacc.Bacc(None, target_bir_lowering=False, num_devices=8)

    xT = nc.declare_dram_parameter("xT", [P, 16, S], bf, isOutput=False)
    wq = nc.declare_dram_parameter("wq", [P, 16, 512], bf, isOutput=False)
    wk = nc.declare_dram_parameter("wk", [P, 16, 512], bf, isOutput=False)
    wv = nc.declare_dram_parameter("wv", [P, 16, 512], bf, isOutput=False)
    wo = nc.declare_dram_parameter("wo", [P, HPC, 16, P], bf, isOutput=False)
    cosk = nc.declare_dram_parameter("cosk", [P, S], bf, isOutput=False)
    sink = nc.declare_dram_parameter("sink", [P, S], bf, isOutput=False)
    maskk = nc.declare_dram_parameter("maskk", [P, P], bf, isOutput=False)
    negtri = nc.declare_dram_parameter("negtri", [P, P], bf, isOutput=False)
    permm = nc.declare_dram_parameter("permm", [P, P], bf, isOutput=False)
    out_sh = nc.declare_dram_parameter("out_sh", [NP, 512, 512], bf,
                                       isOutput=True)

    rs_in = [nc.dram_tensor(f"rs_in{T}", [4 * 512, 512], bf)
             for T in range(NP)]
    rs_r = [t.rearrange("(mg mi p) s -> mg p mi s", p=P, mi=4) for t in rs_in]
    rs_out = [nc.dram_tensor(f"rs_out{T}", [512, 512], bf) for T in range(NP)]

    with tile.TileContext(nc) as tc, ExitStack() as ctx:
        const = ctx.enter_context(tc.tile_pool(name="const", bufs=1))
        kvres = ctx.enter_context(tc.tile_pool(name="kvres", bufs=1))
        xp = ctx.enter_context(tc.tile_pool(name="xp", bufs=2))
        qp = ctx.enter_context(tc.tile_pool(name="qp", bufs=3))
        rp = ctx.enter_context(tc.tile_pool(name="rp", bufs=4))
        tp = ctx.enter_context(tc.tile_pool(name="tp", bufs=9))
        ep = ctx.enter_context(tc.tile_pool(name="ep", bufs=6))
        bp = ctx.enter_context(tc.tile_pool(name="bp", bufs=2))
        dp = ctx.enter_context(tc.tile_pool(name="dp", bufs=3))
        zp = ctx.enter_context(tc.tile_pool(name="zp", bufs=2))
        op_ = ctx.enter_context(tc.tile_pool(name="op", bufs=2))
        pp = ctx.enter_context(tc.tile_pool(name="pp", bufs=3, space="PSUM"))
        sc = ctx.enter_context(tc.tile_pool(name="sc", bufs=2, space="PSUM"))
        zd = ctx.enter_context(tc.tile_pool(name="zd", bufs=3, space="PSUM"))

        wq_sb = const.tile([P, 16, 512], bf, name="wq_sb")
        wk_sb = const.tile([P, 16, 512], bf, name="wk_sb")
        wv_sb = const.tile([P, 16, 512], bf, name="wv_sb")
        wo_sb = const.tile([P, HPC, 16, P], bf, name="wo_sb")
        cos_sb = const.tile([P, S], bf, name="cos_sb")
        sin_sb = const.tile([P, S], bf, name="sin_sb")
        ident_sb = const.tile([P, P], bf, name="ident_sb")
        negtri_sb = const.tile([P, P], bf, name="negtri_sb")
        permm_sb = const.tile([P, P], bf, name="permm_sb")
        ones_sb = const.tile([P, P], bf, name="ones_sb")

        def load_consts():
            # wq/x phase-0 chunks are emitted by proj_phase(0) before this
            for ks in (slice(0, 4), slice(4, 8), slice(8, 12),
                       slice(12, 16)):
                nc.sync.dma_start(out=wk_sb[:, ks, :], in_=wk[:, ks, :])
            nc.sync.dma_start(out=cos_sb, in_=cosk[:, :])
            nc.sync.dma_start(out=sin_sb, in_=sink[:, :])
            nc.sync.dma_start(out=permm_sb, in_=permm[:, :])
            for half in range(2):
                ks = slice(8 * half, 8 * half + 8)
                nc.sync.dma_start(out=wv_sb[:, ks, :], in_=wv[:, ks, :])
            nc.sync.dma_start(out=wo_sb, in_=wo[:, :, :, :])
            nc.sync.dma_start(out=ident_sb, in_=maskk[:, :])
            nc.sync.dma_start(out=negtri_sb, in_=negtri[:, :])
            nc.vector.memset(ones_sb, 1.0)

        # persistent K (feature-major) and V (token-major) per 512-token phase
        k_sbs = [kvres.tile([P, HPC, 512], bf, tag=f"k_sb{T}", name=f"k_sb{T}")
                 for T in range(NP)]
        v_sbs = [kvres.tile([P, 4, 512], bf, tag=f"v_sb{T}", name=f"v_sb{T}")
                 for T in range(NP)]

        q_sbs = {}
        z_sbs = {}

        def proj_phase(T):
            tok = slice(512 * T, 512 * (T + 1))
            x_t = xp.tile([P, 16, 512], bf, tag="x_t", name=f"x_{T}")
            if T == 0:
                # interleave x and wq chunks so projection matmuls can start
                # as early as possible; everything else follows
                for ks in (slice(0, 2), slice(2, 4), slice(4, 8),
                           slice(8, 12), slice(12, 16)):
                    nc.sync.dma_start(out=x_t[:, ks, :], in_=xT[:, ks, tok])
                    nc.sync.dma_start(out=wq_sb[:, ks, :], in_=wq[:, ks, :])
                load_consts()
            else:
                nc.sync.dma_start(out=x_t[:, 0:8, :], in_=xT[:, 0:8, tok])
                nc.sync.dma_start(out=x_t[:, 8:16, :], in_=xT[:, 8:16, tok])

            # ---- Q / K projections; rotations deferred so the PSUM->SBUF
            # evacuation of each head overlaps the next head's matmul group
            q_sb = qp.tile([P, HPC, 512], bf, tag="q_sb", name=f"q_sb{T}")
            q_sbs[T] = q_sb
            pending = []
            for wt_sb, is_q in ((wq_sb, True), (wk_sb, False)):
                for h in range(HPC):
                    ps = pp.tile([P, 512], f32, tag="pp")
                    for kd in range(16):
                        nc.tensor.matmul(ps[:],
                                         lhsT=wt_sb[:, kd, P * h:P * (h + 1)],
                                         rhs=x_t[:, kd, :],
                                         start=(kd == 0), stop=(kd == 15))
                    t = tp.tile([P, 512], bf, tag="t")
                    if is_q:   # fold the 1/sqrt(Dh) score scale into q
                        nc.scalar.mul(t[:], ps[:], float(DH) ** -0.5)
                    else:
                        nc.scalar.copy(t[:], ps[:])
                    pending.append((t, is_q, h))
            for t, is_q, h in pending:
                ps2 = pp.tile([P, 512], f32, tag="pp")
                nc.tensor.matmul(ps2[:], lhsT=permm_sb[:], rhs=t[:],
                                 start=True, stop=True)
                u = rp.tile([P, 512], bf, tag="u")
                nc.vector.tensor_mul(u[:], t[:], cos_sb[:, tok])
                sw = rp.tile([P, 512], bf, tag="sw")
                nc.vector.tensor_mul(sw[:], ps2[:], sin_sb[:, tok])
                dst = q_sb[:, h, :] if is_q else k_sbs[T][:, h, :]
                nc.vector.tensor_add(dst, u[:], sw[:])

            # ---- V projection (token-major) ----
            for tb in range(4):
                psv = pp.tile([P, 512], f32, tag="pp")
                for kd in range(16):
                    nc.tensor.matmul(psv[:],
                                     lhsT=x_t[:, kd, P * tb:P * (tb + 1)],
                                     rhs=wv_sb[:, kd, :],
                                     start=(kd == 0), stop=(kd == 15))
                nc.scalar.copy(v_sbs[T][:, tb, :], psv[:])

        def attn_phase(T):
            q_sb = q_sbs.pop(T)
            z_sb = zp.tile([P, HPC, 512], bf, tag="z_sb", name=f"z_sb{T}")
            nkb = 4 * T + 4
            for h in range(HPC):
                ps_z = zd.tile([P, 512], f32, tag="zd",
                               name=f"ps_z{T}_{h}")
                den = dp.tile([P, 512], bf, tag="den")
                for kb in range(nkb):
                    j = kb - 4 * T
                    qlo = P * j if j > 0 else 0
                    qs = slice(qlo, 512)
                    first, last = (kb == 0), (kb == nkb - 1)
                    ps_s = sc.tile([P, 512], f32, tag="sc")
                    diag = j >= 0
                    nc.tensor.matmul(
                        ps_s[:, qs],
                        lhsT=k_sbs[kb // 4][:, h, P * (kb % 4):P * (kb % 4 + 1)],
                        rhs=q_sb[:, h, qs],
                        start=True, stop=not diag)
                    if diag:  # causal: add -1e9 upper-triangle into scores
                        nc.tensor.matmul(
                            ps_s[:, qlo:qlo + P],
                            lhsT=ident_sb[:], rhs=negtri_sb[:],
                            start=False, stop=True, skip_group_check=True)
                    e = ep.tile([P, 512], bf, tag="e")
                    nc.scalar.activation(e[:, qs], ps_s[:, qs],
                                         mybir.ActivationFunctionType.Exp)
                    nc.tensor.matmul(
                        ps_z[:, qs],
                        lhsT=v_sbs[kb // 4][:, kb % 4, P * h:P * (h + 1)],
                        rhs=e[:, qs],
                        start=first, stop=last, skip_group_check=True)
                    if first:
                        nc.vector.tensor_copy(den[:], e[:])
                    else:
                        nc.vector.tensor_add(den[:, qs], den[:, qs], e[:, qs])
                ps_f = sc.tile([P, 512], f32, tag="sc")
                nc.tensor.matmul(ps_f[:], lhsT=ones_sb[:], rhs=den[:],
                                 start=True, stop=True)
                bc = bp.tile([P, 512], f32, tag="bc")
                nc.vector.reciprocal(bc[:], ps_f[:])
                nc.vector.tensor_mul(z_sb[:, h, :], ps_z[:], bc[:])
            z_sbs[T] = z_sb

        def wo_phase(T):
            z_sb = z_sbs.pop(T)
            for mg in range(4):
                o_sb = op_.tile([P, 4, 512], bf, tag="o_sb")
                for mi in range(4):
                    m = 4 * mg + mi
                    ps_o = pp.tile([P, 512], f32, tag="pp")
                    for kd in range(HPC):
                        nc.tensor.matmul(ps_o[:],
                                         lhsT=wo_sb[:, kd, m, :],
                                         rhs=z_sb[:, kd, :],
                                         start=(kd == 0), stop=(kd == HPC - 1))
                    nc.vector.tensor_copy(o_sb[:, mi, :], ps_o[:])
                nc.sync.dma_start(out=rs_r[T][mg], in_=o_sb[:])
            nc.gpsimd.collective_compute(
                "ReduceScatter", mybir.AluOpType.add, replica_groups=GROUPS,
                ins=[rs_in[T][:, :]], outs=[rs_out[T][:, :]])
            nc.sync.dma_start(out=out_sh[T, :, :], in_=rs_out[T][:, :])

        for T in range(NP):
            proj_phase(T)
            if T >= 1:
                attn_phase(T - 1)
                wo_phase(T - 1)
        attn_phase(NP - 1)
        wo_phase(NP - 1)

    nc.compile()
    return nc


_BUILT = {}


def _get_built(S):
    if S not in _BUILT:
        _BUILT[S] = _build(S)
    return _BUILT[S]


def _bf16(x: np.ndarray) -> np.ndarray:
    return np.ascontiguousarray(x.astype(BF16))


def host_inputs(x, w_qkv, w_o):
    """Build the 8 per-core input maps from full inputs."""
    B, S, D_ = x.shape

    j = np.arange(0, DH, 2, dtype=np.float32) / DH          # (2j)/Dh, j=0..63
    inv_freq = (1.0 / (ROPE_BASE ** j)).astype(np.float32)  # [64]
    t = np.arange(S, dtype=np.float32)
    freqs = np.outer(inv_freq, t)                            # [64, S]
    emb = np.concatenate([freqs, freqs], axis=0)             # [128, S]
    cos_t = _bf16(np.cos(emb))
    sin_t = _bf16(np.sin(emb))
    # rot = R @ q (rotate_half incl. sign); matmul computes lhsT.T @ rhs,
    # so feed R.T: R[d, d+64] = -1 (d<64), R[d, d-64] = +1 (d>=64)
    permm_np = np.zeros((P, P), dtype=np.float32)
    for d_ in range(64):
        permm_np[d_ + 64, d_] = -1.0
        permm_np[d_, d_ + 64] = 1.0
    permm_np = _bf16(permm_np)

    k_idx = np.arange(P)[:, None]
    q_idx = np.arange(P)[None, :]
    mask_np = _bf16(np.eye(P, dtype=np.float32))             # identity lhsT
    negtri_np = _bf16((q_idx < k_idx).astype(np.float32) * -1e9)

    wqkvT = np.asarray(w_qkv, dtype=np.float32).T            # [D, 3D]
    woT_full = np.asarray(w_o, dtype=np.float32).T           # [D(in), D(out)]

    def wslice(r, n):
        ws = wqkvT[:, n * D + 512 * r:n * D + 512 * (r + 1)]  # [D, 512]
        return _bf16(ws.reshape(16, P, 512).transpose(1, 0, 2))

    xTb = [
        _bf16(np.ascontiguousarray(x[b].T).reshape(16, P, S).transpose(1, 0, 2))
        for b in range(B)
    ]

    in_maps = []
    for c in range(8):
        b, r = c // 4, c % 4
        woc = woT_full[512 * r:512 * (r + 1), :]              # [512, D]
        wo_np = _bf16(woc.reshape(HPC, P, 16, P).transpose(1, 0, 2, 3))
        in_maps.append({
            "xT": xTb[b],
            "wq": wslice(r, 0),
            "wk": wslice(r, 1),
            "wv": wslice(r, 2),
            "wo": wo_np,
            "cosk": cos_t, "sink": sin_t,
            "maskk": mask_np, "negtri": negtri_np, "permm": permm_np,
        })
    return in_maps


def assemble(results, B, S):
    NP = S // 512
    out = np.empty((B, S, D), dtype=np.float32)
    for c in range(8):
        b, r = c // 4, c % 4
        sh = np.asarray(results[c]["out_sh"]).astype(np.float32)
        for T in range(NP):
            out[b, 512 * T:512 * (T + 1), 512 * r:512 * (r + 1)] = sh[T].T
    return out


def kernel(x, w_qkv, w_o, _trace=False):
    x = np.asarray(x, dtype=np.float32)
    w_qkv = np.asarray(w_qkv, dtype=np.float32)
    w_o = np.asarray(w_o, dtype=np.float32)
    B, S, _ = x.shape
    nc = _get_built(S)
    in_maps = host_inputs(x, w_qkv, w_o)

    def _run():
        try:
            return run_bass_kernel_spmd(nc, in_maps, list(range(8)),
                                        trace=_trace)
        except ModuleNotFoundError:
            return run_bass_kernel_spmd(nc, in_maps, list(range(8)))

    try:
        res = _run()
    except Exception:
        res = _run()  # transient runtime/readback errors: retry once
    out = assemble(res.results, B, S)
    if _trace:
        return out, res
    return out


# revision 30
# speedup vs baseline: 1.3371x; 1.0551x over previous
"""Causal multi-head attention (B=2, S=2048, D=2048, H=16) on 8 TRN2 cores.

Sharding: core c = (batch b = c//4, head-group r = c%4 -> heads 4r..4r+3).
Per core: project q/k/v for its 4 heads over all tokens (bf16 matmuls, fp32
PSUM), RoPE, exact-causal attention in transposed-score layout (scoresT
[keys, q] via lhsT=k_fm, rhs=q_fm; z[dh, q] via lhsT=v_tokmajor, rhs=e).
Softmax denominator is accumulated on the TensorEngine (ones-matmul) into
the second half of a [128,1024] PSUM tile shared with the z accumulation.
Output projection partials go through a per-phase bf16 ReduceScatter
across the 4 cores of each batch group.

Numerics: bf16 matmul inputs everywhere (fp32 PSUM accumulation), exp on
ACT (no max-subtraction; scores are O(1)), reciprocal in fp32.
"""
import sys

sys.path.insert(0, "/opt/trn_rl_repo")

from contextlib import ExitStack

import ml_dtypes
import numpy as np

import concourse.bass as bass  # noqa: F401  (bass must import before tile)
import concourse.mybir as mybir
import concourse.tile as tile
from concourse import bacc
from concourse.bass_utils import run_bass_kernel_spmd

dt = mybir.dt
BF16 = ml_dtypes.bfloat16
P = 128
D = 2048
N_HEAD = 16
DH = 128
HPC = 4            # heads per core
ROPE_BASE = 10000.0
GROUPS = [[0, 1, 2, 3], [4, 5, 6, 7]]


def _build(S: int):
    NP = S // 512  # token phases
    f32, bf = dt.float32, dt.bfloat16
    nc = bacc.Bacc(None, target_bir_lowering=False, num_devices=8)

    xT = nc.declare_dram_parameter("xT", [P, 16, S], bf, isOutput=False)
    wq = nc.declare_dram_parameter("wq", [P, 16, 512], bf, isOutput=False)
    wk = nc.declare_dram_parameter("wk", [P, 16, 512], bf, isOutput=False)
    wv = nc.declare_dram_parameter("wv", [P, 16, 512], bf, isOutput=False)
    wo = nc.declare_dram_parameter("wo", [P, HPC, 16, P], bf, isOutput=False)
    cosk = nc.declare_dram_parameter("cosk", [P, S], bf, isOutput=False)
    sink = nc.declare_dram_parameter("sink", [P, S], bf, isOutput=False)
    maskk = nc.declare_dram_parameter("maskk", [P, P], bf, isOutput=False)
    negtri = nc.declare_dram_parameter("negtri", [P, P], bf, isOutput=False)
    permm = nc.declare_dram_parameter("permm", [P, P], bf, isOutput=False)
    out_sh = nc.declare_dram_parameter("out_sh", [NP, 512, 512], bf,
                                       isOutput=True)

    rs_in = [nc.dram_tensor(f"rs_in{T}", [4 * 512, 512], bf)
             for T in range(NP)]
    rs_r = [t.rearrange("(mg mi p) s -> mg p mi s", p=P, mi=4) for t in rs_in]
    rs_out = [nc.dram_tensor(f"rs_out{T}", [512, 512], bf) for T in range(NP)]

    with tile.TileContext(nc) as tc, ExitStack() as ctx:
        const = ctx.enter_context(tc.tile_pool(name="const", bufs=1))
        kvres = ctx.enter_context(tc.tile_pool(name="kvres", bufs=1))
        xp = ctx.enter_context(tc.tile_pool(name="xp", bufs=2))
        qp = ctx.enter_context(tc.tile_pool(name="qp", bufs=3))
        rp = ctx.enter_context(tc.tile_pool(name="rp", bufs=4))
        tp = ctx.enter_context(tc.tile_pool(name="tp", bufs=9))
        ep = ctx.enter_context(tc.tile_pool(name="ep", bufs=8))
        bp = ctx.enter_context(tc.tile_pool(name="bp", bufs=2))
        dp = ctx.enter_context(tc.tile_pool(name="dp", bufs=3))
        zp = ctx.enter_context(tc.tile_pool(name="zp", bufs=2))
        op_ = ctx.enter_context(tc.tile_pool(name="op", bufs=2))
        pp = ctx.enter_context(tc.tile_pool(name="pp", bufs=2, space="PSUM"))
        sc = ctx.enter_context(tc.tile_pool(name="sc", bufs=2, space="PSUM"))
        zd = ctx.enter_context(tc.tile_pool(name="zd", bufs=2, space="PSUM"))

        wq_sb = const.tile([P, 16, 512], bf, name="wq_sb")
        wk_sb = const.tile([P, 16, 512], bf, name="wk_sb")
        wv_sb = const.tile([P, 16, 512], bf, name="wv_sb")
        wo_sb = const.tile([P, HPC, 16, P], bf, name="wo_sb")
        cos_sb = const.tile([P, S], bf, name="cos_sb")
        sin_sb = const.tile([P, S], bf, name="sin_sb")
        ident_sb = const.tile([P, P], bf, name="ident_sb")
        negtri_sb = const.tile([P, P], bf, name="negtri_sb")
        permm_sb = const.tile([P, P], bf, name="permm_sb")
        ones_sb = const.tile([P, P], bf, name="ones_sb")

        def load_consts():
            # wq/x phase-0 chunks are emitted by proj_phase(0) before this
            for ks in (slice(0, 4), slice(4, 8), slice(8, 12),
                       slice(12, 16)):
                nc.sync.dma_start(out=wk_sb[:, ks, :], in_=wk[:, ks, :])
            nc.sync.dma_start(out=cos_sb, in_=cosk[:, :])
            nc.sync.dma_start(out=sin_sb, in_=sink[:, :])
            nc.sync.dma_start(out=permm_sb, in_=permm[:, :])
            for half in range(2):
                ks = slice(8 * half, 8 * half + 8)
                nc.sync.dma_start(out=wv_sb[:, ks, :], in_=wv[:, ks, :])
            nc.sync.dma_start(out=wo_sb, in_=wo[:, :, :, :])
            nc.sync.dma_start(out=ident_sb, in_=maskk[:, :])
            nc.sync.dma_start(out=negtri_sb, in_=negtri[:, :])
            nc.vector.memset(ones_sb, 1.0)

        # persistent K (feature-major) and V (token-major) per 512-token phase
        k_sbs = [kvres.tile([P, HPC, 512], bf, tag=f"k_sb{T}", name=f"k_sb{T}")
                 for T in range(NP)]
        v_sbs = [kvres.tile([P, 4, 512], bf, tag=f"v_sb{T}", name=f"v_sb{T}")
                 for T in range(NP)]

        q_sbs = {}
        z_sbs = {}
        x_tiles = {}

        def load_x(T):
            if T in x_tiles or T >= NP:
                return
            tok = slice(512 * T, 512 * (T + 1))
            x_t = xp.tile([P, 16, 512], bf, tag="x_t", name=f"x_{T}")
            nc.sync.dma_start(out=x_t[:, 0:8, :], in_=xT[:, 0:8, tok])
            nc.sync.dma_start(out=x_t[:, 8:16, :], in_=xT[:, 8:16, tok])
            x_tiles[T] = x_t

        def proj_phase(T):
            tok = slice(512 * T, 512 * (T + 1))
            if T == 0:
                x_t = xp.tile([P, 16, 512], bf, tag="x_t", name=f"x_{T}")
                x_tiles[T] = x_t
                # interleave x and wq chunks so projection matmuls can start
                # as early as possible; everything else follows
                for ks in (slice(0, 2), slice(2, 4), slice(4, 8),
                           slice(8, 12), slice(12, 16)):
                    nc.sync.dma_start(out=x_t[:, ks, :], in_=xT[:, ks, tok])
                    nc.sync.dma_start(out=wq_sb[:, ks, :], in_=wq[:, ks, :])
                load_consts()
            else:
                load_x(T)
            x_t = x_tiles.pop(T)
            load_x(T + 1)   # prefetch next phase's activations

            # ---- Q / K projections with RoPE rotations staggered two
            # matmul-groups behind (PSUM evac overlaps the next group, and
            # the rope DVE work overlaps later groups instead of tailing)
            q_sb = qp.tile([P, HPC, 512], bf, tag="q_sb", name=f"q_sb{T}")
            q_sbs[T] = q_sb
            pending = []

            def proj_group(wt_sb, is_q, h):
                ps = pp.tile([P, 512], f32, tag="pp")
                for kd in range(16):
                    nc.tensor.matmul(ps[:],
                                     lhsT=wt_sb[:, kd, P * h:P * (h + 1)],
                                     rhs=x_t[:, kd, :],
                                     start=(kd == 0), stop=(kd == 15))
                t = tp.tile([P, 512], bf, tag="t")
                if is_q:   # fold the 1/sqrt(Dh) score scale into q
                    nc.scalar.mul(t[:], ps[:], float(DH) ** -0.5)
                else:
                    nc.scalar.copy(t[:], ps[:])
                pending.append((t, is_q, h))

            def v_group(tb):
                psv = pp.tile([P, 512], f32, tag="pp")
                for kd in range(16):
                    nc.tensor.matmul(psv[:],
                                     lhsT=x_t[:, kd, P * tb:P * (tb + 1)],
                                     rhs=wv_sb[:, kd, :],
                                     start=(kd == 0), stop=(kd == 15))
                nc.scalar.copy(v_sbs[T][:, tb, :], psv[:])

            def rot_head():
                t, is_q, h = pending.pop(0)
                ps2 = pp.tile([P, 512], f32, tag="pp")
                nc.tensor.matmul(ps2[:], lhsT=permm_sb[:], rhs=t[:],
                                 start=True, stop=True)
                u = rp.tile([P, 512], bf, tag="u")
                nc.vector.tensor_mul(u[:], t[:], cos_sb[:, tok])
                sw = rp.tile([P, 512], bf, tag="sw")
                nc.vector.tensor_mul(sw[:], ps2[:], sin_sb[:, tok])
                dst = q_sb[:, h, :] if is_q else k_sbs[T][:, h, :]
                nc.vector.tensor_add(dst, u[:], sw[:])

            work = [(proj_group, (wt_sb, is_q, h))
                    for wt_sb, is_q in ((wq_sb, True), (wk_sb, False))
                    for h in range(HPC)]
            work += [(v_group, (tb,)) for tb in range(4)]
            for i, (fn, args) in enumerate(work):
                fn(*args)
                # rotations trail ~5 groups behind: their DVE work overlaps
                # the second half of the projection groups + V projection
                if i >= 1 and pending and len(pending) + i >= 9:
                    rot_head()
            while pending:
                rot_head()

        def attn_phase(T):
            q_sb = q_sbs.pop(T)
            z_sb = zp.tile([P, HPC, 512], bf, tag="z_sb", name=f"z_sb{T}")
            nkb = 4 * T + 4
            # off-diagonal key blocks processed in pairs sharing one exp op;
            # diagonal blocks stay single (range-restricted + triangle bias)
            groups = [(2 * i, 2 * i + 1) for i in range(2 * T)]
            groups += [(kb,) for kb in range(4 * T, nkb)]
            for h in range(HPC):
                ps_z = zd.tile([P, 512], f32, tag="zd",
                               name=f"ps_z{T}_{h}")
                den = dp.tile([P, 512], bf, tag="den")
                for g in groups:
                    ps_s = sc.tile([P, 2, 512], f32, tag="sc")
                    e = ep.tile([P, 2, 512], bf, tag="e")
                    for idx, kb in enumerate(g):
                        j = kb - 4 * T
                        qlo = P * j if j > 0 else 0
                        qs = slice(qlo, 512)
                        diag = j >= 0
                        nc.tensor.matmul(
                            ps_s[:, idx, qs],
                            lhsT=k_sbs[kb // 4][:, h,
                                                P * (kb % 4):P * (kb % 4 + 1)],
                            rhs=q_sb[:, h, qs],
                            start=True, stop=not diag)
                        if diag:  # causal: add -1e9 upper triangle
                            nc.tensor.matmul(
                                ps_s[:, idx, qlo:qlo + P],
                                lhsT=ident_sb[:], rhs=negtri_sb[:],
                                start=False, stop=True, skip_group_check=True)
                    if len(g) == 2:
                        nc.scalar.activation(
                            e[:, :, :], ps_s[:, :, :],
                            mybir.ActivationFunctionType.Exp)
                    else:
                        j = g[0] - 4 * T
                        qlo = P * j if j > 0 else 0
                        nc.scalar.activation(
                            e[:, 0, qlo:], ps_s[:, 0, qlo:],
                            mybir.ActivationFunctionType.Exp)
                    for idx, kb in enumerate(g):
                        j = kb - 4 * T
                        qlo = P * j if j > 0 else 0
                        qs = slice(qlo, 512)
                        first, last = (kb == 0), (kb == nkb - 1)
                        nc.tensor.matmul(
                            ps_z[:, qs],
                            lhsT=v_sbs[kb // 4][:, kb % 4,
                                                P * h:P * (h + 1)],
                            rhs=e[:, idx, qs],
                            start=first, stop=last, skip_group_check=True)
                        if first:
                            nc.vector.tensor_copy(den[:], e[:, 0, :])
                        else:
                            nc.vector.tensor_add(den[:, qs], den[:, qs],
                                                 e[:, idx, qs])
                ps_f = sc.tile([P, 512], f32, tag="sc")
                nc.tensor.matmul(ps_f[:], lhsT=ones_sb[:], rhs=den[:],
                                 start=True, stop=True)
                bc = bp.tile([P, 512], f32, tag="bc")
                nc.vector.reciprocal(bc[:], ps_f[:])
                nc.vector.tensor_mul(z_sb[:, h, :], ps_z[:], bc[:])
            z_sbs[T] = z_sb

        def wo_phase(T):
            z_sb = z_sbs.pop(T)
            for mg in range(4):
                o_sb = op_.tile([P, 4, 512], bf, tag="o_sb")
                for mi in range(4):
                    m = 4 * mg + mi
                    ps_o = pp.tile([P, 512], f32, tag="pp")
                    for kd in range(HPC):
                        nc.tensor.matmul(ps_o[:],
                                         lhsT=wo_sb[:, kd, m, :],
                                         rhs=z_sb[:, kd, :],
                                         start=(kd == 0), stop=(kd == HPC - 1))
                    nc.vector.tensor_copy(o_sb[:, mi, :], ps_o[:])
                nc.sync.dma_start(out=rs_r[T][mg], in_=o_sb[:])
            nc.gpsimd.collective_compute(
                "ReduceScatter", mybir.AluOpType.add, replica_groups=GROUPS,
                ins=[rs_in[T][:, :]], outs=[rs_out[T][:, :]])
            nc.sync.dma_start(out=out_sh[T, :, :], in_=rs_out[T][:, :])

        for T in range(NP):
            proj_phase(T)
            if T >= 1:
                attn_phase(T - 1)
                wo_phase(T - 1)
        attn_phase(NP - 1)
        wo_phase(NP - 1)

    nc.compile()
    return nc


_BUILT = {}


def _get_built(S):
    if S not in _BUILT:
        _BUILT[S] = _build(S)
    return _BUILT[S]


def _bf16(x: np.ndarray) -> np.ndarray:
    return np.ascontiguousarray(x.astype(BF16))


def host_inputs(x, w_qkv, w_o):
    """Build the 8 per-core input maps from full inputs."""
    B, S, D_ = x.shape

    j = np.arange(0, DH, 2, dtype=np.float32) / DH          # (2j)/Dh, j=0..63
    inv_freq = (1.0 / (ROPE_BASE ** j)).astype(np.float32)  # [64]
    t = np.arange(S, dtype=np.float32)
    freqs = np.outer(inv_freq, t)                            # [64, S]
    emb = np.concatenate([freqs, freqs], axis=0)             # [128, S]
    cos_t = _bf16(np.cos(emb))
    sin_t = _bf16(np.sin(emb))
    # rot = R @ q (rotate_half incl. sign); matmul computes lhsT.T @ rhs,
    # so feed R.T: R[d, d+64] = -1 (d<64), R[d, d-64] = +1 (d>=64)
    permm_np = np.zeros((P, P), dtype=np.float32)
    for d_ in range(64):
        permm_np[d_ + 64, d_] = -1.0
        permm_np[d_, d_ + 64] = 1.0
    permm_np = _bf16(permm_np)

    k_idx = np.arange(P)[:, None]
    q_idx = np.arange(P)[None, :]
    mask_np = _bf16(np.eye(P, dtype=np.float32))             # identity lhsT
    negtri_np = _bf16((q_idx < k_idx).astype(np.float32) * -1e9)

    wqkvT = np.asarray(w_qkv, dtype=np.float32).T            # [D, 3D]
    woT_full = np.asarray(w_o, dtype=np.float32).T           # [D(in), D(out)]

    def wslice(r, n):
        ws = wqkvT[:, n * D + 512 * r:n * D + 512 * (r + 1)]  # [D, 512]
        return _bf16(ws.reshape(16, P, 512).transpose(1, 0, 2))

    xTb = [
        _bf16(np.ascontiguousarray(x[b].T).reshape(16, P, S).transpose(1, 0, 2))
        for b in range(B)
    ]

    in_maps = []
    for c in range(8):
        b, r = c // 4, c % 4
        woc = woT_full[512 * r:512 * (r + 1), :]              # [512, D]
        wo_np = _bf16(woc.reshape(HPC, P, 16, P).transpose(1, 0, 2, 3))
        in_maps.append({
            "xT": xTb[b],
            "wq": wslice(r, 0),
            "wk": wslice(r, 1),
            "wv": wslice(r, 2),
            "wo": wo_np,
            "cosk": cos_t, "sink": sin_t,
            "maskk": mask_np, "negtri": negtri_np, "permm": permm_np,
        })
    return in_maps


def assemble(results, B, S):
    NP = S // 512
    out = np.empty((B, S, D), dtype=np.float32)
    for c in range(8):
        b, r = c // 4, c % 4
        sh = np.asarray(results[c]["out_sh"]).astype(np.float32)
        for T in range(NP):
            out[b, 512 * T:512 * (T + 1), 512 * r:512 * (r + 1)] = sh[T].T
    return out


def kernel(x, w_qkv, w_o, _trace=False):
    x = np.asarray(x, dtype=np.float32)
    w_qkv = np.asarray(w_qkv, dtype=np.float32)
    w_o = np.asarray(w_o, dtype=np.float32)
    B, S, _ = x.shape
    nc = _get_built(S)
    in_maps = host_inputs(x, w_qkv, w_o)

    def _run():
        try:
            return run_bass_kernel_spmd(nc, in_maps, list(range(8)),
                                        trace=_trace)
        except ModuleNotFoundError:
            return run_bass_kernel_spmd(nc, in_maps, list(range(8)))

    try:
        res = _run()
    except Exception:
        res = _run()  # transient runtime/readback errors: retry once
    out = assemble(res.results, B, S)
    if _trace:
        return out, res
    return out
